# revision 1
# baseline (speedup 1.0000x reference)
"""Bass/Trainium2 kernel for BatchingCostModule:
costs[0, i, j] = 0.5 * ||x[0,i,:] - y[0,j,:]||^2  for x,y [1, 4096, 128] f32.

Computed as costs = 0.5*|x|^2 + 0.5*|y|^2 - x @ y.T.

Sharding: rows of x (N=4096) split across 8 NeuronCores (512 rows each);
y replicated. Each core computes its [512, 4096] slice of the cost matrix.

Host-side prep (cheap, O(N*D)): transpose + negate the x shard, transpose y
(so the contraction dim D=128 lands on SBUF partitions and all device DMAs
are contiguous), and precompute the squared-norm bias vectors split into
bf16 high+low pairs. On device, each [128, 512] output tile is produced by
a matmul (-xT.T @ yT -> -x.y) plus a K=4 bf16 matmul accumulating
0.5|x|^2 + 0.5|y|^2 into the same PSUM bank, then copied PSUM->SBUF and
DMAed out as contiguous 2MB row-blocks.
"""

import os

import numpy as np

N_CORES = 8
B, N, M, D = 1, 4096, 4096, 128
RPC = N // N_CORES  # rows of x per core = 512
NT = 512  # matmul moving free dim / psum bank (fp32)
N_CT = M // NT  # 8 column tiles
N_RT = RPC // 128  # 4 row tiles

# Matmul precision mode:
#   fp32   - exact-ish (PE does 2-pass fp32, 4 cyc/row)
#   fp32r  - single-pass reduced-precision fp32 (1 cyc/row at N>=256)
#   bf16x3 - xh@yh + xh@yl + xl@yh with host-split bf16 pairs (~2^-16 rel err)
#   bf16   - single bf16 matmul (~3e-4 rel err)
MODE = os.environ.get("BK_MODE", "bf16x3")

_CACHE = {}


def _split_bf16(v):
    """v (f64 array) -> (h, l) bf16 arrays with h+l ~= v."""
    import ml_dtypes

    h = v.astype(np.float32).astype(ml_dtypes.bfloat16)
    l = (v - h.astype(np.float64)).astype(np.float32).astype(ml_dtypes.bfloat16)
    return h, l


def _build(mode):
    import concourse.bacc as bacc
    import concourse.bass as bass
    import concourse.mybir as mybir
    import concourse.tile as tile

    f32 = mybir.dt.float32
    f32r = mybir.dt.float32r
    bf16 = mybir.dt.bfloat16

    nc = bacc.Bacc(
        "TRN2", target_bir_lowering=False, debug=False, num_devices=N_CORES
    )

    def din(name, shape, dt_):
        return nc.dram_tensor(name, shape, dt_, kind="ExternalInput").ap()

    main_dt = {"fp32": f32, "fp32r": f32, "bf16x3": bf16, "bf16": bf16}[mode]
    # y chunks pre-sliced on host: [N_CT, 128, NT] so each chunk is contiguous
    yh_d = din("yh", [N_CT, D, NT], main_dt)
    nxh_d = din("nxh", [D, RPC], main_dt)
    if mode == "bf16x3":
        yl_d = din("yl", [N_CT, D, NT], bf16)
        nxl_d = din("nxl", [D, RPC], bf16)
    bl_d = din("bl", [4, RPC], bf16)
    br_d = din("br", [4, M], bf16)
    out_d = nc.dram_tensor("out", [RPC, M], f32, kind="ExternalOutput").ap()

    with tile.TileContext(nc) as tc:
        with (
            tc.tile_pool(name="cst", bufs=1) as cp,
            tc.tile_pool(name="ob", bufs=2) as ob,
            tc.tile_pool(name="ps", bufs=8, space=bass.MemorySpace.PSUM) as pp,
        ):
            ych = []
            ycl = []
            for ct in range(N_CT):
                t = cp.tile([D, NT], main_dt, tag=f"yh{ct}")
                nc.sync.dma_start(t[:], yh_d[ct])
                ych.append(t)
                if mode == "bf16x3":
                    t2 = cp.tile([D, NT], bf16, tag=f"yl{ct}")
                    nc.sync.dma_start(t2[:], yl_d[ct])
                    ycl.append(t2)
            nxh_t = cp.tile([D, RPC], main_dt, tag="nxh")
            nc.sync.dma_start(nxh_t[:], nxh_d[:])
            if mode == "bf16x3":
                nxl_t = cp.tile([D, RPC], bf16, tag="nxl")
                nc.sync.dma_start(nxl_t[:], nxl_d[:])
            bl_t = cp.tile([4, RPC], bf16, tag="bl")
            nc.sync.dma_start(bl_t[:], bl_d[:])
            br_t = cp.tile([4, M], bf16, tag="br")
            nc.sync.dma_start(br_t[:], br_d[:])

            for rt in range(N_RT):
                o = ob.tile([128, M], f32, tag="ob")
                rs = slice(rt * 128, (rt + 1) * 128)
                for ct in range(N_CT):
                    ps = pp.tile([128, NT], f32, tag="ps")
                    cs = slice(ct * NT, (ct + 1) * NT)
                    if mode == "fp32r":
                        nc.tensor.matmul(
                            ps[:],
                            nxh_t[:, rs].bitcast(f32r),
                            ych[ct][:].bitcast(f32r),
                            start=True,
                            stop=False,
                        )
                    else:
                        nc.tensor.matmul(
                            ps[:], nxh_t[:, rs], ych[ct][:],
                            start=True, stop=False,
                        )
                    if mode == "bf16x3":
                        nc.tensor.matmul(
                            ps[:], nxh_t[:, rs], ycl[ct][:],
                            start=False, stop=False,
                        )
                        nc.tensor.matmul(
                            ps[:], nxl_t[:, rs], ych[ct][:],
                            start=False, stop=False,
                        )
                    nc.tensor.matmul(
                        ps[:], bl_t[:, rs], br_t[:, cs],
                        start=False, stop=True,
                    )
                    if ct % 2 == 0:
                        nc.scalar.copy(o[:, cs], ps[:])
                    else:
                        nc.vector.tensor_copy(o[:, cs], ps[:])
                nc.sync.dma_start(out_d[rs, :], o[:])

    nc.compile()
    return nc


def _prep_in_maps(x, y, mode):
    import ml_dtypes

    x = np.asarray(x).reshape(N, D)
    y = np.asarray(y).reshape(M, D)

    x64 = x.astype(np.float64)
    y64 = y.astype(np.float64)
    x2h, x2l = _split_bf16(0.5 * (x64 * x64).sum(-1))  # [N]
    y2h, y2l = _split_bf16(0.5 * (y64 * y64).sum(-1))  # [M]
    ones = np.ones(M, dtype=ml_dtypes.bfloat16)

    yt = np.ascontiguousarray(y.T)  # [D, M]
    ytc = np.ascontiguousarray(yt.reshape(D, N_CT, NT).transpose(1, 0, 2))

    br = np.stack([ones, ones, y2h, y2l])  # [4, M]

    if mode in ("fp32", "fp32r"):
        yh_full = ytc.astype(np.float32)
        yl_full = None
    elif mode == "bf16":
        yh_full = ytc.astype(ml_dtypes.bfloat16)
        yl_full = None
    else:  # bf16x3
        yh_full = ytc.astype(ml_dtypes.bfloat16)
        yl_full = (ytc.astype(np.float64) - yh_full.astype(np.float64)).astype(
            np.float32
        ).astype(ml_dtypes.bfloat16)

    in_maps = []
    for c in range(N_CORES):
        rows = slice(c * RPC, (c + 1) * RPC)
        nxt = np.ascontiguousarray(-x[rows].T)  # [D, RPC] f32
        onesr = np.ones(RPC, dtype=ml_dtypes.bfloat16)
        bl = np.stack([x2h[rows], x2l[rows], onesr, onesr])  # [4, RPC]
        m = {"bl": bl, "br": br}
        if mode in ("fp32", "fp32r"):
            m["yh"] = yh_full
            m["nxh"] = nxt
        elif mode == "bf16":
            m["yh"] = yh_full
            m["nxh"] = nxt.astype(ml_dtypes.bfloat16)
        else:
            nxh = nxt.astype(ml_dtypes.bfloat16)
            nxl = (nxt.astype(np.float64) - nxh.astype(np.float64)).astype(
                np.float32
            ).astype(ml_dtypes.bfloat16)
            m["yh"] = yh_full
            m["yl"] = yl_full
            m["nxh"] = nxh
            m["nxl"] = nxl
        in_maps.append(m)
    return in_maps


LAST_RESULTS = None


def kernel(x, y):
    global LAST_RESULTS
    from concourse.bass_utils import run_bass_kernel_spmd

    mode = MODE
    if mode not in _CACHE:
        _CACHE[mode] = _build(mode)
    nc = _CACHE[mode]

    in_maps = _prep_in_maps(x, y, mode)
    trace = os.environ.get("BK_TRACE", "0") == "1"
    res = run_bass_kernel_spmd(
        nc, in_maps, core_ids=list(range(N_CORES)), trace=trace
    )
    LAST_RESULTS = res

    out = np.empty((B, N, M), dtype=np.float32)
    for c in range(N_CORES):
        out[0, c * RPC : (c + 1) * RPC, :] = res.results[c]["out"]
    return out


# revision 9
# speedup vs baseline: 1.2146x; 1.2146x over previous
"""Bass/Trainium2 kernel for BatchingCostModule:
costs[0, i, j] = 0.5 * ||x[0,i,:] - y[0,j,:]||^2  for x,y [1, 4096, 128] f32.

Computed as costs = 0.5*|x|^2 + 0.5*|y|^2 - x @ y.T.

Sharding: rows of x (N=4096) split across 8 NeuronCores (512 rows each);
y replicated. Each core computes its [512, 4096] slice of the cost matrix.

Host-side prep (cheap, O(N*D)): transpose + negate the x shard, transpose y
(so the contraction dim D=128 lands on SBUF partitions and all device DMAs
are contiguous), and precompute the squared-norm bias vectors split into
bf16 high+low pairs. On device, each [128, 512] output tile is produced by
matmuls (-xT.T @ yT -> -x.y) plus a K=4 bf16 matmul accumulating
0.5|x|^2 + 0.5|y|^2 into the same PSUM bank, then copied PSUM->SBUF and
DMAed out as contiguous 1MB half-row-blocks.

Schedule notes (from NTFF traces): input DMAs are batched into 3
dma_start instructions (SP-engine issue costs ~650ns each); matmuls are
ordered so consecutive matmuls share the stationary operand (8 column
tiles back-to-back per weight load, one PSUM bank each) which keeps PE
streaming at N cycles/matmul instead of paying isolated-matmul drain.
"""

import os

import numpy as np

N_CORES = 8
B, N, M, D = 1, 4096, 4096, 128
RPC = N // N_CORES  # rows of x per core = 512
NT = 512  # matmul moving free dim / psum bank (fp32)
N_CT = M // NT  # 8 column tiles
N_RT = RPC // 128  # 4 row tiles

# Matmul precision mode:
#   fp32   - exact-ish (PE does 2-pass fp32, 4 cyc/row)
#   fp32r  - single-pass reduced-precision fp32 (1 cyc/row at N>=256)
#   bf16x3 - xh@yh + xh@yl + xl@yh with host-split bf16 pairs (~2^-16 rel err)
#   bf16   - single bf16 matmul (~3e-4 rel err)
MODE = os.environ.get("BK_MODE", "bf16x3")

_CACHE = {}


def _split_bf16(v):
    """v (f64 array) -> (h, l) bf16 arrays with h+l ~= v."""
    import ml_dtypes

    h = v.astype(np.float32).astype(ml_dtypes.bfloat16)
    l = (v - h.astype(np.float64)).astype(np.float32).astype(ml_dtypes.bfloat16)
    return h, l


def _round_fp32r(a):
    """Round f32 to the fp32r format: 11-bit mantissa (low 12 bits zero),
    round-to-nearest-even. Same bit layout as f32 otherwise."""
    u = np.ascontiguousarray(a, dtype=np.float32).view(np.uint32).astype(np.uint64)
    u = (u + 0x7FF + ((u >> 12) & 1)) & 0xFFFFF000
    return u.astype(np.uint32).view(np.float32)


def _build(mode):
    import concourse.bacc as bacc
    import concourse.bass as bass
    import concourse.mybir as mybir
    import concourse.tile as tile

    f32 = mybir.dt.float32
    f32r = mybir.dt.float32r
    bf16 = mybir.dt.bfloat16

    nc = bacc.Bacc(
        "TRN2", target_bir_lowering=False, debug=False, num_devices=N_CORES
    )

    def din(name, shape, dt_):
        return nc.dram_tensor(name, shape, dt_, kind="ExternalInput").ap()

    main_dt = {"fp32": f32, "fp32r": f32r, "bf16x3": bf16, "bf16": bf16}[mode]
    n_planes = 2 if mode == "bf16x3" else 1
    # all of y in one DMA: planes (h[, l]) side by side on the free axis
    y_d = din("y", [n_planes, D, M], main_dt)
    # x shard, negated+transposed; planes side by side
    nx_d = din("nx", [n_planes, D, RPC], main_dt)
    # bias rows [x2h, x2l, 1, 1] for cols 0..RPC, [1, 1, y2h, y2l] after
    bias_d = din("bias", [4, RPC + M], bf16)
    out_d = nc.dram_tensor("out", [RPC, M], f32, kind="ExternalOutput").ap()

    with tile.TileContext(nc) as tc:
        with (
            tc.tile_pool(name="cst", bufs=1) as cp,
            tc.tile_pool(name="ob", bufs=2) as ob,
            tc.tile_pool(name="ps", bufs=8, space=bass.MemorySpace.PSUM) as pp,
        ):
            nx_t = cp.tile([D, n_planes * RPC], main_dt, tag="nx")
            nc.sync.dma_start(
                nx_t[:].rearrange("p (c f) -> p c f", c=n_planes),
                nx_d.rearrange("c p f -> p c f"),
            )
            y_t = cp.tile([D, n_planes * M], main_dt, tag="y")
            nc.sync.dma_start(
                y_t[:].rearrange("p (c f) -> p c f", c=n_planes),
                y_d.rearrange("c p f -> p c f"),
            )
            bias_t = cp.tile([4, RPC + M], bf16, tag="bias")
            nc.sync.dma_start(bias_t[:], bias_d[:])

            bl = bias_t[:, 0:RPC]
            br = bias_t[:, RPC : RPC + M]

            for rt in range(N_RT):
                o = ob.tile([128, M], f32, tag="ob")
                rs = slice(rt * 128, (rt + 1) * 128)
                pss = []
                # group 1: stationary nx-high, all 8 column tiles
                for ct in range(N_CT):
                    ps = pp.tile([128, NT], f32, tag="ps")
                    pss.append(ps)
                    nc.tensor.matmul(
                        ps[:],
                        nx_t[:, rs],
                        y_t[:, ct * NT : (ct + 1) * NT],
                        start=True,
                        stop=False,
                    )
                if mode == "bf16x3":
                    # group 2: nx-high @ y-low
                    for ct in range(N_CT):
                        nc.tensor.matmul(
                            pss[ct][:],
                            nx_t[:, rs],
                            y_t[:, M + ct * NT : M + (ct + 1) * NT],
                            start=False,
                            stop=False,
                        )
                    # group 3: nx-low @ y-high
                    for ct in range(N_CT):
                        nc.tensor.matmul(
                            pss[ct][:],
                            nx_t[:, RPC + rt * 128 : RPC + (rt + 1) * 128],
                            y_t[:, ct * NT : (ct + 1) * NT],
                            start=False,
                            stop=False,
                        )
                # bias group + copies chasing it
                for ct in range(N_CT):
                    cs = slice(ct * NT, (ct + 1) * NT)
                    nc.tensor.matmul(
                        pss[ct][:], bl[:, rt * 128 : (rt + 1) * 128], br[:, cs],
                        start=False, stop=True,
                    )
                for ct in range(N_CT):
                    cs = slice(ct * NT, (ct + 1) * NT)
                    if ct % 2 == 0:
                        nc.scalar.copy(o[:, cs], pss[ct][:])
                    else:
                        nc.vector.tensor_copy(o[:, cs], pss[ct][:])
                    if ct == 3:
                        nc.sync.dma_start(
                            out_d[rs, 0 : 4 * NT], o[:, 0 : 4 * NT]
                        )
                nc.sync.dma_start(out_d[rs, 4 * NT : M], o[:, 4 * NT : M])

    nc.compile()
    return nc


def _prep_in_maps(x, y, mode):
    import ml_dtypes

    x = np.asarray(x).reshape(N, D)
    y = np.asarray(y).reshape(M, D)

    x64 = x.astype(np.float64)
    y64 = y.astype(np.float64)
    x2h, x2l = _split_bf16(0.5 * (x64 * x64).sum(-1))  # [N]
    y2h, y2l = _split_bf16(0.5 * (y64 * y64).sum(-1))  # [M]
    ones = np.ones(M, dtype=ml_dtypes.bfloat16)

    yt = np.ascontiguousarray(y.T)  # [D, M]

    if mode == "fp32":
        y_full = yt.astype(np.float32)[None]
    elif mode == "fp32r":
        y_full = _round_fp32r(yt)[None]
    elif mode == "bf16":
        y_full = yt.astype(ml_dtypes.bfloat16)[None]
    else:  # bf16x3
        yh = yt.astype(ml_dtypes.bfloat16)
        yl = (yt.astype(np.float64) - yh.astype(np.float64)).astype(
            np.float32
        ).astype(ml_dtypes.bfloat16)
        y_full = np.stack([yh, yl])
    y_full = np.ascontiguousarray(y_full)

    br = np.stack([ones, ones, y2h, y2l])  # [4, M]

    in_maps = []
    for c in range(N_CORES):
        rows = slice(c * RPC, (c + 1) * RPC)
        nxt = np.ascontiguousarray(-x[rows].T)  # [D, RPC] f32
        onesr = np.ones(RPC, dtype=ml_dtypes.bfloat16)
        bl = np.stack([x2h[rows], x2l[rows], onesr, onesr])  # [4, RPC]
        bias = np.concatenate([bl, br], axis=1)  # [4, RPC + M]
        if mode == "fp32":
            nx = nxt[None]
        elif mode == "fp32r":
            nx = _round_fp32r(nxt)[None]
        elif mode == "bf16":
            nx = nxt.astype(ml_dtypes.bfloat16)[None]
        else:
            nxh = nxt.astype(ml_dtypes.bfloat16)
            nxl = (nxt.astype(np.float64) - nxh.astype(np.float64)).astype(
                np.float32
            ).astype(ml_dtypes.bfloat16)
            nx = np.stack([nxh, nxl])
        in_maps.append(
            {"y": y_full, "nx": np.ascontiguousarray(nx), "bias": bias}
        )
    return in_maps


LAST_RESULTS = None


def kernel(x, y):
    global LAST_RESULTS
    from concourse.bass_utils import run_bass_kernel_spmd

    mode = MODE
    if mode not in _CACHE:
        _CACHE[mode] = _build(mode)
    nc = _CACHE[mode]

    in_maps = _prep_in_maps(x, y, mode)
    trace = os.environ.get("BK_TRACE", "0") == "1"
    res = run_bass_kernel_spmd(
        nc, in_maps, core_ids=list(range(N_CORES)), trace=trace
    )
    LAST_RESULTS = res

    out = np.empty((B, N, M), dtype=np.float32)
    for c in range(N_CORES):
        out[0, c * RPC : (c + 1) * RPC, :] = res.results[c]["out"]
    return out


# revision 12
# speedup vs baseline: 1.3746x; 1.1317x over previous
"""Bass/Trainium2 kernel for BatchingCostModule:
costs[0, i, j] = 0.5 * ||x[0,i,:] - y[0,j,:]||^2  for x,y [1, 4096, 128] f32.

Computed as costs = 0.5*|x|^2 + 0.5*|y|^2 - x @ y.T.

Sharding: rows of x (N=4096) split across 8 NeuronCores (512 rows each);
y replicated. Each core computes its [512, 4096] slice of the cost matrix.

Host-side prep (cheap, O(N*D)): transpose + negate the x shard, transpose y
(so the contraction dim D=128 lands on SBUF partitions and all device DMAs
are contiguous), and precompute the squared-norm bias vectors split into
bf16 high+low pairs. On device, each [128, 512] output tile is produced by
matmuls (-xT.T @ yT -> -x.y) plus a K=4 bf16 matmul accumulating
0.5|x|^2 + 0.5|y|^2 into the same PSUM bank, then copied PSUM->SBUF
(alternating ScalarE/VectorE) and DMAed out as contiguous 1MB row-block
halves.

Schedule notes (from NTFF traces):
- dma_start issue costs ~650ns of SP-engine time, so inputs are batched
  into 6 instructions; y is split into 4 chunk tiles so the first matmuls
  start as soon as the first quarter of y has landed.
- matmuls are ordered so consecutive matmuls share the stationary operand
  (8 column tiles back-to-back per weight load, one PSUM bank each) which
  keeps PE streaming at ~245ns/matmul instead of paying isolated-matmul
  drain (~370ns).
"""

import os

import numpy as np

N_CORES = 8
B, N, M, D = 1, 4096, 4096, 128
RPC = N // N_CORES  # rows of x per core = 512
NT = 512  # matmul moving free dim / psum bank (fp32)
N_CT = M // NT  # 8 column tiles
N_RT = RPC // 128  # 4 row tiles
YC = 1024  # y chunk width
N_YC = M // YC  # 4 y chunks per plane

# Matmul precision mode (error = max|err| / max|costs| measured vs fp32 ref):
#   fp32   - exact-ish (PE does 2-pass fp32, 4 cyc/row; slowest)
#   fp32r  - single-pass fp32 with 11-bit mantissa (~1.8e-4)
#   bf16x3 - xh@yh + xh@yl + xl@yh with host-split bf16 pairs (~5e-6)
#   fp16   - single fp16 matmul (~9e-5), half the PE time of bf16x3
#   bf16   - single bf16 matmul (~7e-4)
MODE = os.environ.get("BK_MODE", "bf16x3")

_CACHE = {}


def _split_bf16(v):
    """v (f64 array) -> (h, l) bf16 arrays with h+l ~= v."""
    import ml_dtypes

    h = v.astype(np.float32).astype(ml_dtypes.bfloat16)
    l = (v - h.astype(np.float64)).astype(np.float32).astype(ml_dtypes.bfloat16)
    return h, l


def _round_fp32r(a):
    """Round f32 to the fp32r format: 11-bit mantissa (low 12 bits zero),
    round-to-nearest-even. Same bit layout as f32 otherwise."""
    u = np.ascontiguousarray(a, dtype=np.float32).view(np.uint32).astype(np.uint64)
    u = (u + 0x7FF + ((u >> 12) & 1)) & 0xFFFFF000
    return u.astype(np.uint32).view(np.float32)


def _build(mode):
    import concourse.bacc as bacc
    import concourse.bass as bass
    import concourse.mybir as mybir
    import concourse.tile as tile

    f32 = mybir.dt.float32
    f32r = mybir.dt.float32r
    bf16 = mybir.dt.bfloat16
    fp16 = mybir.dt.float16

    nc = bacc.Bacc(
        "TRN2", target_bir_lowering=False, debug=False, num_devices=N_CORES
    )

    def din(name, shape, dt_):
        return nc.dram_tensor(name, shape, dt_, kind="ExternalInput").ap()

    main_dt = {
        "fp32": f32, "fp32r": f32r, "bf16x3": bf16, "bf16": bf16, "fp16": fp16
    }[mode]
    n_planes = 2 if mode == "bf16x3" else 1
    # y chunked on host: plane-major, 4 contiguous [D, YC] chunks per plane
    y_d = din("y", [n_planes, N_YC, D, YC], main_dt)
    nx_d = din("nx", [n_planes, D, RPC], main_dt)
    # bias rows [x2h, x2l, 1, 1] for cols 0..RPC, [1, 1, y2h, y2l] after
    bias_d = din("bias", [4, RPC + M], bf16)
    out_d = nc.dram_tensor("out", [RPC, M], f32, kind="ExternalOutput").ap()

    with tile.TileContext(nc) as tc:
        with (
            tc.tile_pool(name="cst", bufs=1) as cp,
            tc.tile_pool(name="ob", bufs=2) as ob,
            tc.tile_pool(name="ps", bufs=8, space=bass.MemorySpace.PSUM) as pp,
        ):
            # issue order: y0, nx, y1.., yl chunks, bias (bias is only
            # needed at the first bias matmul, ~10us in)
            ych = [[None] * N_YC for _ in range(n_planes)]
            t0 = cp.tile([D, YC], main_dt, tag="y_0_0")
            nc.sync.dma_start(t0[:], y_d[0, 0])
            ych[0][0] = t0
            nx_t = cp.tile([D, n_planes * RPC], main_dt, tag="nx")
            nc.sync.dma_start(
                nx_t[:].rearrange("p (c f) -> p c f", c=n_planes),
                nx_d.rearrange("c p f -> p c f"),
            )
            for pl in range(n_planes):
                for g in range(N_YC):
                    if pl == 0 and g == 0:
                        continue
                    t = cp.tile([D, YC], main_dt, tag=f"y_{pl}_{g}")
                    nc.sync.dma_start(t[:], y_d[pl, g])
                    ych[pl][g] = t
            bias_t = cp.tile([4, RPC + M], bf16, tag="bias")
            nc.sync.dma_start(bias_t[:], bias_d[:])

            def yslice(pl, ct):
                c = (ct % (YC // NT)) * NT
                return ych[pl][ct // (YC // NT)][:, c : c + NT]

            bl = bias_t[:, 0:RPC]
            br = bias_t[:, RPC : RPC + M]

            # half-phases: 4 column tiles per weight group, so PSUM->SBUF
            # copies and output DMA chunks are spread evenly through the
            # kernel instead of bursting 2MB at each row-block boundary.
            HC = N_CT // 2
            for rt in range(N_RT):
                rs = slice(rt * 128, (rt + 1) * 128)
                for half in range(2):
                    o = ob.tile([128, HC * NT], f32, tag="ob")
                    cts = range(half * HC, (half + 1) * HC)
                    pss = {}
                    for ct in cts:
                        ps = pp.tile([128, NT], f32, tag="ps")
                        pss[ct] = ps
                        nc.tensor.matmul(
                            ps[:], nx_t[:, rs], yslice(0, ct),
                            start=True, stop=False,
                        )
                    if mode == "bf16x3":
                        for ct in cts:
                            nc.tensor.matmul(
                                pss[ct][:], nx_t[:, rs], yslice(1, ct),
                                start=False, stop=False,
                            )
                        for ct in cts:
                            nc.tensor.matmul(
                                pss[ct][:],
                                nx_t[:, RPC + rt * 128 : RPC + (rt + 1) * 128],
                                yslice(0, ct),
                                start=False, stop=False,
                            )
                    for ct in cts:
                        cs = slice(ct * NT, (ct + 1) * NT)
                        nc.tensor.matmul(
                            pss[ct][:],
                            bl[:, rt * 128 : (rt + 1) * 128], br[:, cs],
                            start=False, stop=True,
                        )
                    for ct in cts:
                        co = slice((ct - half * HC) * NT, (ct - half * HC + 1) * NT)
                        if ct % 2 == 0:
                            nc.scalar.copy(o[:, co], pss[ct][:])
                        else:
                            nc.vector.tensor_copy(o[:, co], pss[ct][:])
                    cso = slice(half * HC * NT, (half + 1) * HC * NT)
                    nc.sync.dma_start(out_d[rs, cso], o[:])

    nc.compile()
    return nc


def _prep_in_maps(x, y, mode):
    import ml_dtypes

    x = np.asarray(x).reshape(N, D)
    y = np.asarray(y).reshape(M, D)

    x64 = x.astype(np.float64)
    y64 = y.astype(np.float64)
    x2h, x2l = _split_bf16(0.5 * (x64 * x64).sum(-1))  # [N]
    y2h, y2l = _split_bf16(0.5 * (y64 * y64).sum(-1))  # [M]
    ones = np.ones(M, dtype=ml_dtypes.bfloat16)

    yt = np.ascontiguousarray(y.T)  # [D, M]

    if mode == "fp32":
        y_pl = yt.astype(np.float32)[None]
    elif mode == "fp32r":
        y_pl = _round_fp32r(yt)[None]
    elif mode == "fp16":
        y_pl = yt.astype(np.float16)[None]
    elif mode == "bf16":
        y_pl = yt.astype(ml_dtypes.bfloat16)[None]
    else:  # bf16x3
        yh = yt.astype(ml_dtypes.bfloat16)
        yl = (yt.astype(np.float64) - yh.astype(np.float64)).astype(
            np.float32
        ).astype(ml_dtypes.bfloat16)
        y_pl = np.stack([yh, yl])
    # [n_planes, N_YC, D, YC] contiguous chunks
    y_full = np.ascontiguousarray(
        y_pl.reshape(-1, D, N_YC, YC).transpose(0, 2, 1, 3)
    )

    br = np.stack([ones, ones, y2h, y2l])  # [4, M]

    in_maps = []
    for c in range(N_CORES):
        rows = slice(c * RPC, (c + 1) * RPC)
        nxt = np.ascontiguousarray(-x[rows].T)  # [D, RPC] f32
        onesr = np.ones(RPC, dtype=ml_dtypes.bfloat16)
        bl = np.stack([x2h[rows], x2l[rows], onesr, onesr])  # [4, RPC]
        bias = np.concatenate([bl, br], axis=1)  # [4, RPC + M]
        if mode == "fp32":
            nx = nxt[None]
        elif mode == "fp32r":
            nx = _round_fp32r(nxt)[None]
        elif mode == "fp16":
            nx = nxt.astype(np.float16)[None]
        elif mode == "bf16":
            nx = nxt.astype(ml_dtypes.bfloat16)[None]
        else:
            nxh = nxt.astype(ml_dtypes.bfloat16)
            nxl = (nxt.astype(np.float64) - nxh.astype(np.float64)).astype(
                np.float32
            ).astype(ml_dtypes.bfloat16)
            nx = np.stack([nxh, nxl])
        in_maps.append(
            {"y": y_full, "nx": np.ascontiguousarray(nx), "bias": bias}
        )
    return in_maps


LAST_RESULTS = None


def kernel(x, y):
    global LAST_RESULTS
    from concourse.bass_utils import run_bass_kernel_spmd

    mode = MODE
    if mode not in _CACHE:
        _CACHE[mode] = _build(mode)
    nc = _CACHE[mode]

    in_maps = _prep_in_maps(x, y, mode)
    trace = os.environ.get("BK_TRACE", "0") == "1"
    res = run_bass_kernel_spmd(
        nc, in_maps, core_ids=list(range(N_CORES)), trace=trace
    )
    LAST_RESULTS = res

    out = np.empty((B, N, M), dtype=np.float32)
    for c in range(N_CORES):
        out[0, c * RPC : (c + 1) * RPC, :] = res.results[c]["out"]
    return out


# revision 13
# speedup vs baseline: 1.5349x; 1.1166x over previous
"""Bass/Trainium2 kernel for BatchingCostModule:
costs[0, i, j] = 0.5 * ||x[0,i,:] - y[0,j,:]||^2  for x,y [1, 4096, 128] f32.

Computed as costs = 0.5*|x|^2 + 0.5*|y|^2 - x @ y.T.

Sharding: rows of x (N=4096) split across 8 NeuronCores (512 rows each);
y replicated. Each core computes its [512, 4096] slice of the cost matrix.

Device algorithm (mode bf16x3, the default): x and y are split on the host
into bf16 high+low pairs (xh+xl ~= x to ~2^-17). Each [128, 512] output
tile accumulates three matmuls in PSUM:
    g1 = (-xh).T @ yh      g2 = (-xg2).T @ yl'      g3 = (-xl).T @ yh
where g2 donates contraction rows 126-127: xg2 rows 126/127 are -1 and
yl' rows 126/127 carry a bf16 high/low split of 0.5*|y_j|^2 (so the y^2
bias term rides the matmul for free; the two dropped xh*yl correction
terms are ~2^-9 scale, error ~1e-5 relative). The 0.5*|x_i|^2 term is
added during the PSUM->SBUF copy (ScalarE activation bias / VectorE
tensor_scalar_add, per-partition f32). Result tiles stream out as 1MB
contiguous DMA chunks.

Schedule notes (from NTFF traces):
- dma_start issue costs ~650ns of SP time and input wire rate is
  ~350GB/s, so inputs are packed into 6 DMAs ordered by first use; the
  first packed DMA carries all of x plus the first y chunk so matmuls
  start ~9.5us in (entry barrier + engine init occupy the first ~7us).
- matmuls are ordered so consecutive matmuls share the stationary
  operand (4 column tiles back-to-back per weight load = one half
  row-block), keeping PE at ~245ns/matmul; half-row-block phases spread
  the PSUM->SBUF copies and output DMA evenly through the kernel.
- 8 PSUM banks double-buffer the two half-phases in flight.

Host-side prep is O(N*D) marshaling: transpose/negate/split x, transpose/
split y (contraction dim D=128 on SBUF partitions makes every device DMA
contiguous), squared norms in f64.
"""

import os

import numpy as np

N_CORES = 8
B, N, M, D = 1, 4096, 4096, 128
RPC = N // N_CORES  # rows of x per core = 512
NT = 512  # matmul moving free dim / psum bank (fp32)
N_CT = M // NT  # 8 column tiles
N_RT = RPC // 128  # 4 row tiles
YC = 1024  # y chunk width
N_YC = M // YC  # 4 y chunks per plane
HC = N_CT // 2  # column tiles per half-phase

# Matmul precision mode (error = max|err| / max|costs| measured vs fp32 ref):
#   bf16x3 - three bf16 matmuls per tile (~1.1e-5), the default
#   fp32r  - single-pass fp32 with 11-bit mantissa (~1.8e-4)
#   fp32   - exact-ish 2-pass fp32 (slowest)
#   fp16   - single fp16 matmul (~9e-5; fp16 streams 2x slower than bf16)
#   bf16   - single bf16 matmul (~7e-4)
MODE = os.environ.get("BK_MODE", "bf16x3")

_CACHE = {}


def _split_bf16(v):
    """v (f64 array) -> (h, l) bf16 arrays with h+l ~= v."""
    import ml_dtypes

    h = v.astype(np.float32).astype(ml_dtypes.bfloat16)
    l = (v - h.astype(np.float64)).astype(np.float32).astype(ml_dtypes.bfloat16)
    return h, l


def _round_fp32r(a):
    """Round f32 to the fp32r format: 11-bit mantissa (low 12 bits zero),
    round-to-nearest-even. Same bit layout as f32 otherwise."""
    u = np.ascontiguousarray(a, dtype=np.float32).view(np.uint32).astype(np.uint64)
    u = (u + 0x7FF + ((u >> 12) & 1)) & 0xFFFFF000
    return u.astype(np.uint32).view(np.float32)


def _build_bf16x3(nc, bass, mybir, tile):
    f32 = mybir.dt.float32
    bf16 = mybir.dt.bfloat16

    def din(name, shape, dt_):
        return nc.dram_tensor(name, shape, dt_, kind="ExternalInput").ap()

    # packed inputs, ordered by first use on device:
    #   p0 = [nxh | nxg2 | nxl | yh0]   p1 = [yh1 | yl0]   p2 = [yl1 | yh2]
    #   p3 = [yh3 | yl2]                p4 = [yl3]
    # (yh_g = bf16 high plane of y.T columns g*1024..; yl_g = low plane with
    # rows 126/127 replaced by the 0.5*|y|^2 bf16 high/low rows; nxg2 = nxh
    # with rows 126/127 = -1... see module docstring)
    p_shapes = [3 * RPC + YC, 2 * YC, 2 * YC, 2 * YC, YC]
    p_d = [din(f"p{i}", [D, w], bf16) for i, w in enumerate(p_shapes)]
    x2_d = din("x2", [128, N_RT], f32)
    out_d = nc.dram_tensor("out", [RPC, M], f32, kind="ExternalOutput").ap()

    with tile.TileContext(nc) as tc:
        with (
            tc.tile_pool(name="cst", bufs=1) as cp,
            tc.tile_pool(name="ob", bufs=3) as ob,
            tc.tile_pool(name="ps", bufs=8, space=bass.MemorySpace.PSUM) as pp,
        ):
            p_t = []
            for i, w in enumerate(p_shapes):
                t = cp.tile([D, w], bf16, tag=f"p{i}")
                nc.sync.dma_start(t[:], p_d[i][:])
                p_t.append(t)
            x2_t = cp.tile([128, N_RT], f32, tag="x2")
            nc.sync.dma_start(x2_t[:], x2_d[:])

            nxh = p_t[0][:, 0:RPC]
            nxg2 = p_t[0][:, RPC : 2 * RPC]
            nxl = p_t[0][:, 2 * RPC : 3 * RPC]
            # (tile, col offset) of each 1024-wide y chunk, per plane
            ychunk = {
                (0, 0): (p_t[0], 3 * RPC),
                (0, 1): (p_t[1], 0),
                (1, 0): (p_t[1], YC),
                (1, 1): (p_t[2], 0),
                (0, 2): (p_t[2], YC),
                (0, 3): (p_t[3], 0),
                (1, 2): (p_t[3], YC),
                (1, 3): (p_t[4], 0),
            }

            def yslice(pl, ct):
                t, off = ychunk[(pl, ct // (YC // NT))]
                c = off + (ct % (YC // NT)) * NT
                return t[:, c : c + NT]

            for rt in range(N_RT):
                rs = slice(rt * 128, (rt + 1) * 128)
                x2col = x2_t[:, rt : rt + 1]
                for half in range(2):
                    o = ob.tile([128, HC * NT], f32, tag="ob")
                    cts = range(half * HC, (half + 1) * HC)
                    pss = {}
                    for ct in cts:
                        ps = pp.tile([128, NT], f32, tag="ps")
                        pss[ct] = ps
                        nc.tensor.matmul(
                            ps[:], nxh[:, rs], yslice(0, ct),
                            start=True, stop=False,
                        )
                    for ct in cts:
                        nc.tensor.matmul(
                            pss[ct][:], nxg2[:, rs], yslice(1, ct),
                            start=False, stop=False,
                        )
                    for ct in cts:
                        nc.tensor.matmul(
                            pss[ct][:], nxl[:, rs], yslice(0, ct),
                            start=False, stop=True,
                        )
                    for ct in cts:
                        co = slice(
                            (ct - half * HC) * NT, (ct - half * HC + 1) * NT
                        )
                        if ct % 2 == 0:
                            nc.scalar.add(o[:, co], pss[ct][:], x2col)
                        else:
                            nc.vector.tensor_scalar_add(
                                o[:, co], pss[ct][:], x2col
                            )
                    # stream out; split the very last chunk to shorten the
                    # kernel tail
                    base = half * HC * NT
                    if rt == N_RT - 1 and half == 1:
                        h = HC * NT // 2
                        nc.sync.dma_start(
                            out_d[rs, base : base + h], o[:, 0:h]
                        )
                        nc.sync.dma_start(
                            out_d[rs, base + h : base + 2 * h], o[:, h : 2 * h]
                        )
                    else:
                        nc.sync.dma_start(
                            out_d[rs, base : base + HC * NT], o[:]
                        )
    return ["out"]


def _prep_bf16x3(x, y):
    import ml_dtypes

    bf16 = ml_dtypes.bfloat16
    x = np.asarray(x).reshape(N, D)
    y = np.asarray(y).reshape(M, D)
    x64 = x.astype(np.float64)
    y64 = y.astype(np.float64)
    y2h, y2l = _split_bf16(0.5 * (y64 * y64).sum(-1))  # [M]
    x2 = (0.5 * (x64 * x64).sum(-1)).astype(np.float32)  # [N]

    yt = np.ascontiguousarray(y.T)  # [D, M]
    yh = yt.astype(bf16)
    yl = (yt.astype(np.float64) - yh.astype(np.float64)).astype(
        np.float32
    ).astype(bf16)
    # donate rows 126/127 of the low plane to the y^2 bias
    yl[D - 2] = y2h
    yl[D - 1] = y2l

    yhc = [np.ascontiguousarray(yh[:, g * YC : (g + 1) * YC]) for g in range(N_YC)]
    ylc = [np.ascontiguousarray(yl[:, g * YC : (g + 1) * YC]) for g in range(N_YC)]

    in_maps = []
    for c in range(N_CORES):
        rows = slice(c * RPC, (c + 1) * RPC)
        nxt = -x[rows].T  # [D, RPC] f32
        nxh = nxt.astype(bf16)
        nxl = (nxt.astype(np.float64) - nxh.astype(np.float64)).astype(
            np.float32
        ).astype(bf16)
        nxg2 = nxh.copy()
        nxg2[D - 2] = bf16(1.0)
        nxg2[D - 1] = bf16(1.0)
        p0 = np.ascontiguousarray(
            np.concatenate([nxh, nxg2, nxl, yhc[0]], axis=1)
        )
        p1 = np.ascontiguousarray(np.concatenate([yhc[1], ylc[0]], axis=1))
        p2 = np.ascontiguousarray(np.concatenate([ylc[1], yhc[2]], axis=1))
        p3 = np.ascontiguousarray(np.concatenate([yhc[3], ylc[2]], axis=1))
        p4 = ylc[3]
        x2p = np.ascontiguousarray(
            x2[rows].reshape(N_RT, 128).T
        )  # [128, N_RT]
        in_maps.append(
            {"p0": p0, "p1": p1, "p2": p2, "p3": p3, "p4": p4, "x2": x2p}
        )
    return in_maps


# ---------------------------------------------------------------------------
# generic fallback modes (fp32 / fp32r / fp16 / bf16): one main matmul plane
# plus a K=4 bf16 bias matmul per tile
# ---------------------------------------------------------------------------


def _build_generic(nc, bass, mybir, tile, mode):
    f32 = mybir.dt.float32
    bf16 = mybir.dt.bfloat16
    main_dt = {
        "fp32": f32, "fp32r": mybir.dt.float32r,
        "fp16": mybir.dt.float16, "bf16": bf16,
    }[mode]

    def din(name, shape, dt_):
        return nc.dram_tensor(name, shape, dt_, kind="ExternalInput").ap()

    y_d = din("y", [N_YC, D, YC], main_dt)
    nx_d = din("nx", [D, RPC], main_dt)
    bias_d = din("bias", [4, RPC + M], bf16)
    out_d = nc.dram_tensor("out", [RPC, M], f32, kind="ExternalOutput").ap()

    with tile.TileContext(nc) as tc:
        with (
            tc.tile_pool(name="cst", bufs=1) as cp,
            tc.tile_pool(name="ob", bufs=3) as ob,
            tc.tile_pool(name="ps", bufs=8, space=bass.MemorySpace.PSUM) as pp,
        ):
            ych = []
            for g in range(N_YC):
                t = cp.tile([D, YC], main_dt, tag=f"y{g}")
                nc.sync.dma_start(t[:], y_d[g])
                ych.append(t)
                if g == 0:
                    nx_t = cp.tile([D, RPC], main_dt, tag="nx")
                    nc.sync.dma_start(nx_t[:], nx_d[:])
            bias_t = cp.tile([4, RPC + M], bf16, tag="bias")
            nc.sync.dma_start(bias_t[:], bias_d[:])
            bl = bias_t[:, 0:RPC]
            br = bias_t[:, RPC : RPC + M]

            def yslice(ct):
                c = (ct % (YC // NT)) * NT
                return ych[ct // (YC // NT)][:, c : c + NT]

            for rt in range(N_RT):
                rs = slice(rt * 128, (rt + 1) * 128)
                for half in range(2):
                    o = ob.tile([128, HC * NT], f32, tag="ob")
                    cts = range(half * HC, (half + 1) * HC)
                    pss = {}
                    for ct in cts:
                        ps = pp.tile([128, NT], f32, tag="ps")
                        pss[ct] = ps
                        nc.tensor.matmul(
                            ps[:], nx_t[:, rs], yslice(ct),
                            start=True, stop=False,
                        )
                    for ct in cts:
                        cs = slice(ct * NT, (ct + 1) * NT)
                        nc.tensor.matmul(
                            pss[ct][:],
                            bl[:, rt * 128 : (rt + 1) * 128], br[:, cs],
                            start=False, stop=True,
                        )
                    for ct in cts:
                        co = slice(
                            (ct - half * HC) * NT, (ct - half * HC + 1) * NT
                        )
                        if ct % 2 == 0:
                            nc.scalar.copy(o[:, co], pss[ct][:])
                        else:
                            nc.vector.tensor_copy(o[:, co], pss[ct][:])
                    base = half * HC * NT
                    nc.sync.dma_start(out_d[rs, base : base + HC * NT], o[:])
    return ["out"]


def _prep_generic(x, y, mode):
    import ml_dtypes

    x = np.asarray(x).reshape(N, D)
    y = np.asarray(y).reshape(M, D)
    x64 = x.astype(np.float64)
    y64 = y.astype(np.float64)
    x2h, x2l = _split_bf16(0.5 * (x64 * x64).sum(-1))
    y2h, y2l = _split_bf16(0.5 * (y64 * y64).sum(-1))
    ones = np.ones(M, dtype=ml_dtypes.bfloat16)

    yt = np.ascontiguousarray(y.T)
    cast = {
        "fp32": lambda a: a.astype(np.float32),
        "fp32r": _round_fp32r,
        "fp16": lambda a: a.astype(np.float16),
        "bf16": lambda a: a.astype(ml_dtypes.bfloat16),
    }[mode]
    y_full = np.ascontiguousarray(
        cast(yt).reshape(D, N_YC, YC).transpose(1, 0, 2)
    )
    br = np.stack([ones, ones, y2h, y2l])

    in_maps = []
    for c in range(N_CORES):
        rows = slice(c * RPC, (c + 1) * RPC)
        nx = np.ascontiguousarray(cast(-x[rows].T))
        onesr = np.ones(RPC, dtype=ml_dtypes.bfloat16)
        bl = np.stack([x2h[rows], x2l[rows], onesr, onesr])
        bias = np.ascontiguousarray(np.concatenate([bl, br], axis=1))
        in_maps.append({"y": y_full, "nx": nx, "bias": bias})
    return in_maps


def _build(mode):
    import concourse.bacc as bacc
    import concourse.bass as bass
    import concourse.mybir as mybir
    import concourse.tile as tile

    nc = bacc.Bacc(
        "TRN2", target_bir_lowering=False, debug=False, num_devices=N_CORES
    )
    if mode == "bf16x3":
        _build_bf16x3(nc, bass, mybir, tile)
    else:
        _build_generic(nc, bass, mybir, tile, mode)
    nc.compile()
    return nc


LAST_RESULTS = None


def kernel(x, y):
    global LAST_RESULTS
    from concourse.bass_utils import run_bass_kernel_spmd

    mode = MODE
    if mode not in _CACHE:
        _CACHE[mode] = _build(mode)
    nc = _CACHE[mode]

    if mode == "bf16x3":
        in_maps = _prep_bf16x3(x, y)
    else:
        in_maps = _prep_generic(x, y, mode)
    trace = os.environ.get("BK_TRACE", "0") == "1"
    res = run_bass_kernel_spmd(
        nc, in_maps, core_ids=list(range(N_CORES)), trace=trace
    )
    LAST_RESULTS = res

    out = np.empty((B, N, M), dtype=np.float32)
    for c in range(N_CORES):
        out[0, c * RPC : (c + 1) * RPC, :] = res.results[c]["out"]
    return out
